# revision 6
# baseline (speedup 1.0000x reference)
"""Trainium2 kernel for one beam_step (nn_CaptionModel): fused top-k + beam gather.

Problem (B=32, K=bdash=5, T=12, V=50257, H=512):
  - cand = beam_logprobs_sum[:,:,None] + logprobs.reshape(B,K,V); per batch row
    take the top-5 of the flattened (K*V) candidates.
  - Gather/concat the big per-beam history tensors along the beam dim with the
    selected beam indices, append the current step, and gather the state.

The memory-bound work (>99.9% of traffic: beam_seq_logprobs 386MB read +
beam_seq_logprobs_new 418MB write + unaug_logprobs + state) runs on 8
NeuronCores, data-parallel over the batch dim (4 batch rows per core), as a
single Bass/Tile SPMD program. Each core performs the data-dependent beam
gather with gpsimd indirect-DMA (DRAM->SBUF, offsets from a per-core int32
index tensor) followed by static HWDGE stores (SBUF->DRAM), streamed in ~4MB
chunks across 120 SBUF partitions.

The top-k / index-selection stage is O(B*K*V) compare work with a 5-element
result per row — negligible traffic next to the 870MB gather. It is computed
with the *same eager jax.lax.top_k op the reference uses*, so the selected
indices (including any backend-specific tie/padding semantics of top_k on the
current default jax device) match the reference bit-for-bit. The tiny outputs
(beam_seq_new ~13KB, beam_logprobs_sum_new 640B) are likewise produced by the
same jnp expressions as the reference; only the two large gathers
(beam_seq_logprobs_new, state_new) are offloaded to the Bass kernel.
"""

import sys

if "/opt/trn_rl_repo" not in sys.path:
    sys.path.insert(0, "/opt/trn_rl_repo")

import numpy as np

# Problem shape (hardcoded per spec nn_CaptionModel_47098611368401).
B, K, T, V, H = 32, 5, 12, 50257, 512
BDASH = 5
M = 8                     # cores
BC = B // M               # batch rows per core (4)
R = BC * K                # output rows per core (20)
GSPLIT = 6                # sub-rows per gathered beam row (12*V = 6 * GLEN)
GLEN = T * V // GSPLIT    # 100514
USPLIT = 29               # sub-rows per unaug row (V = 29 * 1733)
ULEN = V // USPLIT        # 1733
UPASS_ROWS = 4            # output rows per unaug pass (4*29=116 partitions)
NUPASS = R // UPASS_ROWS  # 5 passes
CHUNK = 8192              # elements per partition per bslp chunk

_nc_cache = None
last_results = None


def _build_device_program():
    """Bass/Tile program: indirect-gather bslp/unaug/state rows, store to out.

    Per-core I/O (shapes chosen so every gathered unit is one DRAM row):
      bslp  [R*GSPLIT, GLEN] f32 : core's beam_seq_logprobs slice, rows are
                                   sixths of one (b,k) history [T*V].
      unaug [R*USPLIT, ULEN] f32 : core's unaug_logprobs slice, rows are
                                   29ths of one (b,k) vocab row [V].
      state [2*R, H] f32         : core's state slice, flattened (layer,row).
      idxg  [R*GSPLIT, 1] i32    : source row in bslp for each gathered sub-row.
      idxu  [UPASS_ROWS*USPLIT, NUPASS] i32 : per-pass source rows in unaug.
      idxs  [2*R, 1] i32         : source row in state for each output row.
      out   [R, 13*V] f32        : beam_seq_logprobs_new slice (row-major
                                   [bdash concat over b][T+1, V]).
      outs  [2*R, H] f32         : state_new slice.
    """
    global _nc_cache
    if _nc_cache is not None:
        return _nc_cache

    import concourse.bacc as bacc
    import concourse.bass as bass
    import concourse.mybir as mybir
    from concourse.tile import TileContext

    nc = bacc.Bacc("TRN2", target_bir_lowering=False)
    f32, i32 = mybir.dt.float32, mybir.dt.int32
    bslp = nc.dram_tensor("bslp", [R * GSPLIT, GLEN], f32, kind="ExternalInput")
    unaug = nc.dram_tensor("unaug", [R * USPLIT, ULEN], f32, kind="ExternalInput")
    state = nc.dram_tensor("state", [2 * R, H], f32, kind="ExternalInput")
    idxg = nc.dram_tensor("idxg", [R * GSPLIT, 1], i32, kind="ExternalInput")
    idxu = nc.dram_tensor("idxu", [UPASS_ROWS * USPLIT, NUPASS], i32, kind="ExternalInput")
    idxs = nc.dram_tensor("idxs", [2 * R, 1], i32, kind="ExternalInput")
    out = nc.dram_tensor("out", [R, (T + 1) * V], f32, kind="ExternalOutput")
    outs = nc.dram_tensor("outs", [2 * R, H], f32, kind="ExternalOutput")

    # out columns [0, T*V) viewed as [R, GSPLIT, GLEN] for chunked stores.
    outg3 = out[:, : T * V].rearrange("r (s c) -> r s c", c=GLEN)

    # One tile shape per pool: mixing tile shapes (differing partition
    # counts) in a single pool produced device-unrecoverable DMA crashes.
    with TileContext(nc) as tc:
        with (
            tc.tile_pool(name="gpool", bufs=4) as gpool,
            tc.tile_pool(name="upool", bufs=2) as upool,
            tc.tile_pool(name="ig", bufs=1) as igpool,
            tc.tile_pool(name="iu", bufs=1) as iupool,
            tc.tile_pool(name="is", bufs=1) as ispool,
            tc.tile_pool(name="st", bufs=1) as stpool,
        ):
            idxg_sb = igpool.tile([R * GSPLIT, 1], i32)
            nc.sync.dma_start(out=idxg_sb[:], in_=idxg[:])
            idxu_sb = iupool.tile([UPASS_ROWS * USPLIT, NUPASS], i32)
            nc.sync.dma_start(out=idxu_sb[:], in_=idxu[:])
            idxs_sb = ispool.tile([2 * R, 1], i32)
            nc.sync.dma_start(out=idxs_sb[:], in_=idxs[:])

            # state gather: 40 rows of H floats.
            st = stpool.tile([2 * R, H], f32)
            nc.gpsimd.indirect_dma_start(
                out=st[:], out_offset=None, in_=state[:],
                in_offset=bass.IndirectOffsetOnAxis(ap=idxs_sb[:, :1], axis=0),
            )
            nc.sync.dma_start(out=outs[:], in_=st[:])

            # beam_seq_logprobs gather: 120 sub-rows x GLEN, chunked.
            for lo in range(0, GLEN, CHUNK):
                hi = min(GLEN, lo + CHUNK)
                w = hi - lo
                tile = gpool.tile([R * GSPLIT, CHUNK], f32, tag="g")
                nc.gpsimd.indirect_dma_start(
                    out=tile[:, :w], out_offset=None, in_=bslp[:],
                    in_offset=bass.IndirectOffsetOnAxis(ap=idxg_sb[:, :1], axis=0),
                    element_offset=lo,
                )
                nc.sync.dma_start(out=outg3[:, :, lo:hi], in_=tile[:, :w])

            # unaug gather -> out columns [T*V, (T+1)*V), 4 output rows per pass.
            for p in range(NUPASS):
                tu = upool.tile([UPASS_ROWS * USPLIT, ULEN], f32, tag="u")
                nc.gpsimd.indirect_dma_start(
                    out=tu[:], out_offset=None, in_=unaug[:],
                    in_offset=bass.IndirectOffsetOnAxis(ap=idxu_sb[:, p : p + 1], axis=0),
                )
                r0 = p * UPASS_ROWS
                nc.sync.dma_start(
                    out=out[r0 : r0 + UPASS_ROWS, T * V :], in_=tu[:]
                )
    nc.finalize()
    _nc_cache = nc
    return nc


def _reference_jax(logprobs, unaug_logprobs, beam_seq, beam_seq_logprobs,
                   beam_logprobs_sum, state, bdash):
    """Full eager-jax replication of the reference (fallback for unexpected
    problem instances)."""
    import jax
    import jax.numpy as jnp

    batch = beam_logprobs_sum.shape[0]
    vocab = logprobs.shape[-1]
    lp = jnp.asarray(logprobs).reshape(batch, -1, vocab)
    n_beams = lp.shape[1]
    cand = jnp.asarray(beam_logprobs_sum)[:, :, None] + lp
    flat = cand.reshape(batch, -1)
    ys, ix = jax.lax.top_k(flat, bdash)
    beam_ix = ix // vocab
    selected_ix = ix % vocab
    state_ix = (beam_ix + jnp.arange(batch)[:, None] * n_beams).reshape(-1)
    beam_seq_j = jnp.asarray(beam_seq)
    beam_seq_g = jnp.take_along_axis(beam_seq_j, beam_ix[:, :, None], axis=1)
    beam_seq_new = jnp.concatenate(
        [beam_seq_g, selected_ix[:, :, None].astype(beam_seq_j.dtype)], axis=-1)
    beam_logprobs_sum_new = (
        jnp.take_along_axis(jnp.asarray(beam_logprobs_sum), beam_ix, axis=1)
        + jnp.take_along_axis(lp.reshape(batch, -1), ix, axis=1))
    bslp_g = jnp.take_along_axis(
        jnp.asarray(beam_seq_logprobs), beam_ix[:, :, None, None], axis=1)
    beam_lp = jnp.take_along_axis(
        jnp.asarray(unaug_logprobs).reshape(batch, -1, vocab),
        beam_ix[:, :, None], axis=1)
    beam_seq_logprobs_new = jnp.concatenate(
        [bslp_g, beam_lp[:, :, None, :]], axis=2)
    state_new = jnp.asarray(state)[:, state_ix]
    return (np.asarray(beam_seq_new), np.asarray(beam_seq_logprobs_new),
            np.asarray(beam_logprobs_sum_new), np.asarray(state_new))


def kernel(logprobs, unaug_logprobs, beam_seq, beam_seq_logprobs,
           beam_logprobs_sum, state, bdash):
    logprobs = np.asarray(logprobs)
    unaug_logprobs = np.asarray(unaug_logprobs)
    beam_seq = np.asarray(beam_seq)
    beam_seq_logprobs = np.asarray(beam_seq_logprobs)
    beam_logprobs_sum = np.asarray(beam_logprobs_sum)
    state = np.asarray(state)
    bdash = int(bdash)

    expected_shape = (
        bdash == BDASH
        and logprobs.shape == (B * K, V)
        and unaug_logprobs.shape == (B * K, V)
        and beam_seq_logprobs.shape == (B, K, T, V)
        and beam_logprobs_sum.shape == (B, K)
        and state.shape[1:] == (B * K, H)
    )
    if not expected_shape:
        return _reference_jax(logprobs, unaug_logprobs, beam_seq,
                              beam_seq_logprobs, beam_logprobs_sum, state, bdash)

    # ---- selection + tiny outputs: the reference's own ops, verbatim -----
    # Using the identical eager jax.lax.top_k (on the process-default jax
    # device) guarantees the selected indices match the reference even where
    # the backend's top_k lowering has nonstandard index/tie semantics.
    import jax
    import jax.numpy as jnp

    lp = jnp.asarray(logprobs).reshape(B, K, V)
    sums_j = jnp.asarray(beam_logprobs_sum)
    cand = sums_j[:, :, None] + lp
    flat = cand.reshape(B, -1)
    _ys, ix = jax.lax.top_k(flat, BDASH)
    beam_ix = ix // V
    selected_ix = ix % V
    beam_seq_j = jnp.asarray(beam_seq)
    beam_seq_g = jnp.take_along_axis(beam_seq_j, beam_ix[:, :, None], axis=1)
    beam_seq_new = jnp.concatenate(
        [beam_seq_g, selected_ix[:, :, None].astype(beam_seq_j.dtype)], axis=-1)
    beam_logprobs_sum_new = (
        jnp.take_along_axis(sums_j, beam_ix, axis=1)
        + jnp.take_along_axis(lp.reshape(B, -1), ix, axis=1))

    # XLA gathers clamp out-of-range indices; replicate for the device gather.
    beam_ix_np = np.clip(np.asarray(beam_ix).astype(np.int64), 0, K - 1)

    # ---- device: all heavy gathers --------------------------------------
    from concourse.bass_utils import run_bass_kernel_spmd

    nc = _build_device_program()

    src_local = (np.arange(BC)[:, None] * K + beam_ix_np.reshape(M, BC, BDASH)
                 ).astype(np.int32)                    # [M, BC, 5] in [0, R)
    sub_g = np.arange(GSPLIT, dtype=np.int32)
    sub_u = np.arange(USPLIT, dtype=np.int32)

    in_maps = []
    for m in range(M):
        sl = src_local[m].reshape(R)                   # [20]
        idxg = (sl[:, None] * GSPLIT + sub_g[None, :]).reshape(-1, 1)
        idxu = ((sl[:, None] * USPLIT + sub_u[None, :])
                .reshape(NUPASS, UPASS_ROWS * USPLIT).T.copy())
        idxs = np.concatenate([sl, R + sl]).reshape(-1, 1)
        in_maps.append({
            "bslp": beam_seq_logprobs[m * BC:(m + 1) * BC].reshape(R * GSPLIT, GLEN),
            "unaug": unaug_logprobs[m * R:(m + 1) * R].reshape(R * USPLIT, ULEN),
            "state": state[:, m * R:(m + 1) * R, :].reshape(2 * R, H),
            "idxg": np.ascontiguousarray(idxg),
            "idxu": np.ascontiguousarray(idxu),
            "idxs": np.ascontiguousarray(idxs),
        })

    import os
    trace = bool(os.environ.get("KERNEL_TRACE"))
    res = run_bass_kernel_spmd(nc, in_maps, core_ids=list(range(M)), trace=trace)
    global last_results
    last_results = res

    beam_seq_logprobs_new = np.concatenate(
        [r["out"].reshape(BC, BDASH, T + 1, V) for r in res.results], axis=0)
    state_new = np.concatenate(
        [r["outs"].reshape(2, R, H) for r in res.results], axis=1)

    return (np.asarray(beam_seq_new), beam_seq_logprobs_new,
            np.asarray(beam_logprobs_sum_new), state_new)


# revision 7
# speedup vs baseline: 1.1800x; 1.1800x over previous
"""Trainium2 kernel for one beam_step (nn_CaptionModel): fused top-k + beam gather.

Problem (B=32, K=bdash=5, T=12, V=50257, H=512):
  - cand = beam_logprobs_sum[:,:,None] + logprobs.reshape(B,K,V); per batch row
    take the top-5 of the flattened (K*V) candidates.
  - Gather/concat the big per-beam history tensors along the beam dim with the
    selected beam indices, append the current step, and gather the state.

The memory-bound work (>99.9% of traffic: beam_seq_logprobs 386MB read +
beam_seq_logprobs_new 418MB write + unaug_logprobs + state) runs on 8
NeuronCores, data-parallel over the batch dim (4 batch rows per core), as a
single Bass/Tile SPMD program. Each core performs the data-dependent beam
gather with gpsimd indirect-DMA (DRAM->SBUF, offsets from a per-core int32
index tensor) followed by static HWDGE stores (SBUF->DRAM), streamed in ~4MB
chunks across 120 SBUF partitions.

The top-k / index-selection stage is O(B*K*V) compare work with a 5-element
result per row — negligible traffic next to the 870MB gather. It is computed
with the *same eager jax.lax.top_k op the reference uses*, so the selected
indices (including any backend-specific tie/padding semantics of top_k on the
current default jax device) match the reference bit-for-bit. The tiny outputs
(beam_seq_new ~13KB, beam_logprobs_sum_new 640B) are likewise produced by the
same jnp expressions as the reference; only the two large gathers
(beam_seq_logprobs_new, state_new) are offloaded to the Bass kernel.
"""

import sys

if "/opt/trn_rl_repo" not in sys.path:
    sys.path.insert(0, "/opt/trn_rl_repo")

import numpy as np

# Problem shape (hardcoded per spec nn_CaptionModel_47098611368401).
B, K, T, V, H = 32, 5, 12, 50257, 512
BDASH = 5
M = 8                     # cores
BC = B // M               # batch rows per core (4)
R = BC * K                # output rows per core (20)
GSPLIT = 6                # sub-rows per gathered beam row (12*V = 6 * GLEN)
GLEN = T * V // GSPLIT    # 100514
USPLIT = 29               # sub-rows per unaug row (V = 29 * 1733)
ULEN = V // USPLIT        # 1733
UPASS_ROWS = 4            # output rows per unaug pass (4*29=116 partitions)
NUPASS = R // UPASS_ROWS  # 5 passes
import os as _os
CHUNK = int(_os.environ.get("KCHUNK", "8192"))   # elements/partition per bslp chunk
KBUFS = int(_os.environ.get("KBUFS", "4"))       # gpool double-buffering depth
KUBUFS = int(_os.environ.get("KUBUFS", "2"))     # upool depth

_nc_cache = None
last_results = None


def _build_device_program():
    """Bass/Tile program: indirect-gather bslp/unaug/state rows, store to out.

    Per-core I/O (shapes chosen so every gathered unit is one DRAM row):
      bslp  [R*GSPLIT, GLEN] f32 : core's beam_seq_logprobs slice, rows are
                                   sixths of one (b,k) history [T*V].
      unaug [R*USPLIT, ULEN] f32 : core's unaug_logprobs slice, rows are
                                   29ths of one (b,k) vocab row [V].
      state [2*R, H] f32         : core's state slice, flattened (layer,row).
      idxg  [R*GSPLIT, 1] i32    : source row in bslp for each gathered sub-row.
      idxu  [UPASS_ROWS*USPLIT, NUPASS] i32 : per-pass source rows in unaug.
      idxs  [2*R, 1] i32         : source row in state for each output row.
      out   [R, 13*V] f32        : beam_seq_logprobs_new slice (row-major
                                   [bdash concat over b][T+1, V]).
      outs  [2*R, H] f32         : state_new slice.
    """
    global _nc_cache
    if _nc_cache is not None:
        return _nc_cache

    import concourse.bacc as bacc
    import concourse.bass as bass
    import concourse.mybir as mybir
    from concourse.tile import TileContext

    nc = bacc.Bacc("TRN2", target_bir_lowering=False)
    f32, i32 = mybir.dt.float32, mybir.dt.int32
    bslp = nc.dram_tensor("bslp", [R * GSPLIT, GLEN], f32, kind="ExternalInput")
    unaug = nc.dram_tensor("unaug", [R * USPLIT, ULEN], f32, kind="ExternalInput")
    state = nc.dram_tensor("state", [2 * R, H], f32, kind="ExternalInput")
    idxg = nc.dram_tensor("idxg", [R * GSPLIT, 1], i32, kind="ExternalInput")
    idxu = nc.dram_tensor("idxu", [UPASS_ROWS * USPLIT, NUPASS], i32, kind="ExternalInput")
    idxs = nc.dram_tensor("idxs", [2 * R, 1], i32, kind="ExternalInput")
    out = nc.dram_tensor("out", [R, (T + 1) * V], f32, kind="ExternalOutput")
    outs = nc.dram_tensor("outs", [2 * R, H], f32, kind="ExternalOutput")

    # out columns [0, T*V) viewed as [R, GSPLIT, GLEN] for chunked stores.
    outg3 = out[:, : T * V].rearrange("r (s c) -> r s c", c=GLEN)

    # One tile shape per pool: mixing tile shapes (differing partition
    # counts) in a single pool produced device-unrecoverable DMA crashes.
    with TileContext(nc) as tc:
        with (
            tc.tile_pool(name="gpool", bufs=KBUFS) as gpool,
            tc.tile_pool(name="upool", bufs=KUBUFS) as upool,
            tc.tile_pool(name="ig", bufs=1) as igpool,
            tc.tile_pool(name="iu", bufs=1) as iupool,
            tc.tile_pool(name="is", bufs=1) as ispool,
            tc.tile_pool(name="st", bufs=1) as stpool,
        ):
            idxg_sb = igpool.tile([R * GSPLIT, 1], i32)
            nc.sync.dma_start(out=idxg_sb[:], in_=idxg[:])
            idxu_sb = iupool.tile([UPASS_ROWS * USPLIT, NUPASS], i32)
            nc.sync.dma_start(out=idxu_sb[:], in_=idxu[:])
            idxs_sb = ispool.tile([2 * R, 1], i32)
            nc.sync.dma_start(out=idxs_sb[:], in_=idxs[:])

            # state gather: 40 rows of H floats.
            st = stpool.tile([2 * R, H], f32)
            nc.gpsimd.indirect_dma_start(
                out=st[:], out_offset=None, in_=state[:],
                in_offset=bass.IndirectOffsetOnAxis(ap=idxs_sb[:, :1], axis=0),
            )
            nc.sync.dma_start(out=outs[:], in_=st[:])

            # beam_seq_logprobs gather: 120 sub-rows x GLEN, chunked.
            for lo in range(0, GLEN, CHUNK):
                hi = min(GLEN, lo + CHUNK)
                w = hi - lo
                tile = gpool.tile([R * GSPLIT, CHUNK], f32, tag="g")
                nc.gpsimd.indirect_dma_start(
                    out=tile[:, :w], out_offset=None, in_=bslp[:],
                    in_offset=bass.IndirectOffsetOnAxis(ap=idxg_sb[:, :1], axis=0),
                    element_offset=lo,
                )
                nc.sync.dma_start(out=outg3[:, :, lo:hi], in_=tile[:, :w])

            # unaug gather -> out columns [T*V, (T+1)*V), 4 output rows per pass.
            for p in range(NUPASS):
                tu = upool.tile([UPASS_ROWS * USPLIT, ULEN], f32, tag="u")
                nc.gpsimd.indirect_dma_start(
                    out=tu[:], out_offset=None, in_=unaug[:],
                    in_offset=bass.IndirectOffsetOnAxis(ap=idxu_sb[:, p : p + 1], axis=0),
                )
                r0 = p * UPASS_ROWS
                nc.sync.dma_start(
                    out=out[r0 : r0 + UPASS_ROWS, T * V :], in_=tu[:]
                )
    nc.finalize()
    _nc_cache = nc
    return nc


def _reference_jax(logprobs, unaug_logprobs, beam_seq, beam_seq_logprobs,
                   beam_logprobs_sum, state, bdash):
    """Full eager-jax replication of the reference (fallback for unexpected
    problem instances)."""
    import jax
    import jax.numpy as jnp

    batch = beam_logprobs_sum.shape[0]
    vocab = logprobs.shape[-1]
    lp = jnp.asarray(logprobs).reshape(batch, -1, vocab)
    n_beams = lp.shape[1]
    cand = jnp.asarray(beam_logprobs_sum)[:, :, None] + lp
    flat = cand.reshape(batch, -1)
    ys, ix = jax.lax.top_k(flat, bdash)
    beam_ix = ix // vocab
    selected_ix = ix % vocab
    state_ix = (beam_ix + jnp.arange(batch)[:, None] * n_beams).reshape(-1)
    beam_seq_j = jnp.asarray(beam_seq)
    beam_seq_g = jnp.take_along_axis(beam_seq_j, beam_ix[:, :, None], axis=1)
    beam_seq_new = jnp.concatenate(
        [beam_seq_g, selected_ix[:, :, None].astype(beam_seq_j.dtype)], axis=-1)
    beam_logprobs_sum_new = (
        jnp.take_along_axis(jnp.asarray(beam_logprobs_sum), beam_ix, axis=1)
        + jnp.take_along_axis(lp.reshape(batch, -1), ix, axis=1))
    bslp_g = jnp.take_along_axis(
        jnp.asarray(beam_seq_logprobs), beam_ix[:, :, None, None], axis=1)
    beam_lp = jnp.take_along_axis(
        jnp.asarray(unaug_logprobs).reshape(batch, -1, vocab),
        beam_ix[:, :, None], axis=1)
    beam_seq_logprobs_new = jnp.concatenate(
        [bslp_g, beam_lp[:, :, None, :]], axis=2)
    state_new = jnp.asarray(state)[:, state_ix]
    return (np.asarray(beam_seq_new), np.asarray(beam_seq_logprobs_new),
            np.asarray(beam_logprobs_sum_new), np.asarray(state_new))


def kernel(logprobs, unaug_logprobs, beam_seq, beam_seq_logprobs,
           beam_logprobs_sum, state, bdash):
    logprobs = np.asarray(logprobs)
    unaug_logprobs = np.asarray(unaug_logprobs)
    beam_seq = np.asarray(beam_seq)
    beam_seq_logprobs = np.asarray(beam_seq_logprobs)
    beam_logprobs_sum = np.asarray(beam_logprobs_sum)
    state = np.asarray(state)
    bdash = int(bdash)

    expected_shape = (
        bdash == BDASH
        and logprobs.shape == (B * K, V)
        and unaug_logprobs.shape == (B * K, V)
        and beam_seq_logprobs.shape == (B, K, T, V)
        and beam_logprobs_sum.shape == (B, K)
        and state.shape[1:] == (B * K, H)
    )
    if not expected_shape:
        return _reference_jax(logprobs, unaug_logprobs, beam_seq,
                              beam_seq_logprobs, beam_logprobs_sum, state, bdash)

    # ---- selection + tiny outputs: the reference's own ops, verbatim -----
    # Using the identical eager jax.lax.top_k (on the process-default jax
    # device) guarantees the selected indices match the reference even where
    # the backend's top_k lowering has nonstandard index/tie semantics.
    import jax
    import jax.numpy as jnp

    lp = jnp.asarray(logprobs).reshape(B, K, V)
    sums_j = jnp.asarray(beam_logprobs_sum)
    cand = sums_j[:, :, None] + lp
    flat = cand.reshape(B, -1)
    _ys, ix = jax.lax.top_k(flat, BDASH)
    beam_ix = ix // V
    selected_ix = ix % V
    beam_seq_j = jnp.asarray(beam_seq)
    beam_seq_g = jnp.take_along_axis(beam_seq_j, beam_ix[:, :, None], axis=1)
    beam_seq_new = jnp.concatenate(
        [beam_seq_g, selected_ix[:, :, None].astype(beam_seq_j.dtype)], axis=-1)
    beam_logprobs_sum_new = (
        jnp.take_along_axis(sums_j, beam_ix, axis=1)
        + jnp.take_along_axis(lp.reshape(B, -1), ix, axis=1))

    # XLA gathers clamp out-of-range indices; replicate for the device gather.
    beam_ix_np = np.clip(np.asarray(beam_ix).astype(np.int64), 0, K - 1)

    # ---- device: all heavy gathers --------------------------------------
    from concourse.bass_utils import run_bass_kernel_spmd

    nc = _build_device_program()

    src_local = (np.arange(BC)[:, None] * K + beam_ix_np.reshape(M, BC, BDASH)
                 ).astype(np.int32)                    # [M, BC, 5] in [0, R)
    sub_g = np.arange(GSPLIT, dtype=np.int32)
    sub_u = np.arange(USPLIT, dtype=np.int32)

    in_maps = []
    for m in range(M):
        sl = src_local[m].reshape(R)                   # [20]
        idxg = (sl[:, None] * GSPLIT + sub_g[None, :]).reshape(-1, 1)
        idxu = ((sl[:, None] * USPLIT + sub_u[None, :])
                .reshape(NUPASS, UPASS_ROWS * USPLIT).T.copy())
        idxs = np.concatenate([sl, R + sl]).reshape(-1, 1)
        in_maps.append({
            "bslp": beam_seq_logprobs[m * BC:(m + 1) * BC].reshape(R * GSPLIT, GLEN),
            "unaug": unaug_logprobs[m * R:(m + 1) * R].reshape(R * USPLIT, ULEN),
            "state": state[:, m * R:(m + 1) * R, :].reshape(2 * R, H),
            "idxg": np.ascontiguousarray(idxg),
            "idxu": np.ascontiguousarray(idxu),
            "idxs": np.ascontiguousarray(idxs),
        })

    import os
    trace = bool(os.environ.get("KERNEL_TRACE"))
    res = run_bass_kernel_spmd(nc, in_maps, core_ids=list(range(M)), trace=trace)
    global last_results
    last_results = res

    beam_seq_logprobs_new = np.concatenate(
        [r["out"].reshape(BC, BDASH, T + 1, V) for r in res.results], axis=0)
    state_new = np.concatenate(
        [r["outs"].reshape(2, R, H) for r in res.results], axis=1)

    return (np.asarray(beam_seq_new), beam_seq_logprobs_new,
            np.asarray(beam_logprobs_sum_new), state_new)


# revision 10
# speedup vs baseline: 1.5138x; 1.2828x over previous
"""Trainium2 kernel for one beam_step (nn_CaptionModel): fused top-k + beam gather.

Problem (B=32, K=bdash=5, T=12, V=50257, H=512):
  - cand = beam_logprobs_sum[:,:,None] + logprobs.reshape(B,K,V); per batch row
    take the top-5 of the flattened (K*V) candidates.
  - Gather/concat the big per-beam history tensors along the beam dim with the
    selected beam indices, append the current step, and gather the state.

The memory-bound work (>99.9% of traffic: beam_seq_logprobs 386MB read +
beam_seq_logprobs_new 418MB write + unaug_logprobs + state) runs on 8
NeuronCores, data-parallel over the batch dim (4 batch rows per core), as a
single Bass/Tile SPMD program. Each core performs the data-dependent beam
gather with gpsimd indirect-DMA (DRAM->SBUF, offsets from a per-core int32
index tensor) followed by static HWDGE stores (SBUF->DRAM), streamed in ~4MB
chunks across 120 SBUF partitions.

The top-k / index-selection stage is O(B*K*V) compare work with a 5-element
result per row — negligible traffic next to the 870MB gather. It is computed
with the *same eager jax.lax.top_k op the reference uses*, so the selected
indices (including any backend-specific tie/padding semantics of top_k on the
current default jax device) match the reference bit-for-bit. The tiny outputs
(beam_seq_new ~13KB, beam_logprobs_sum_new 640B) are likewise produced by the
same jnp expressions as the reference; only the two large gathers
(beam_seq_logprobs_new, state_new) are offloaded to the Bass kernel.
"""

import sys

if "/opt/trn_rl_repo" not in sys.path:
    sys.path.insert(0, "/opt/trn_rl_repo")

import numpy as np

# Problem shape (hardcoded per spec nn_CaptionModel_47098611368401).
B, K, T, V, H = 32, 5, 12, 50257, 512
BDASH = 5
M = 8                     # cores
BC = B // M               # batch rows per core (4)
R = BC * K                # output rows per core (20)
GSPLIT = 6                # sub-rows per gathered beam row (12*V = 6 * GLEN)
GLEN = T * V // GSPLIT    # 100514
USPLIT = 29               # sub-rows per unaug row (V = 29 * 1733)
ULEN = V // USPLIT        # 1733
UPASS_ROWS = 4            # output rows per unaug pass (4*29=116 partitions)
NUPASS = R // UPASS_ROWS  # 5 passes
import os as _os
CHUNK = int(_os.environ.get("KCHUNK", "8192"))   # elements/partition per bslp chunk
KBUFS = int(_os.environ.get("KBUFS", "4"))       # gpool double-buffering depth
KUBUFS = int(_os.environ.get("KUBUFS", "2"))     # upool depth

KIMPL = _os.environ.get("KIMPL", "hwdge")         # "hwdge" | "indirect"
GCP = 1024                # v3: elements/partition, big bslp chunks ([128, GCP])
NGC = T * V // (128 * GCP)        # 4 full chunks of 131072
GREM = T * V - NGC * 128 * GCP    # 78796 remainder
GR_P, GR_C = 128, 615             # remainder chunk [128, 615] = 78720
GR_TAIL = GREM - GR_P * GR_C      # 76
UCP = 392                 # v3: unaug chunk [128, 392] = 50176
U_TAIL = V - 128 * UCP            # 81
KGBUFS = int(_os.environ.get("KGBUFS", "16"))     # v3 gpool depth

_nc_cache = {}
last_results = None


def _build_device_program():
    """Bass/Tile program: indirect-gather bslp/unaug/state rows, store to out.

    Per-core I/O (shapes chosen so every gathered unit is one DRAM row):
      bslp  [R*GSPLIT, GLEN] f32 : core's beam_seq_logprobs slice, rows are
                                   sixths of one (b,k) history [T*V].
      unaug [R*USPLIT, ULEN] f32 : core's unaug_logprobs slice, rows are
                                   29ths of one (b,k) vocab row [V].
      state [2*R, H] f32         : core's state slice, flattened (layer,row).
      idxg  [R*GSPLIT, 1] i32    : source row in bslp for each gathered sub-row.
      idxu  [UPASS_ROWS*USPLIT, NUPASS] i32 : per-pass source rows in unaug.
      idxs  [2*R, 1] i32         : source row in state for each output row.
      out   [R, 13*V] f32        : beam_seq_logprobs_new slice (row-major
                                   [bdash concat over b][T+1, V]).
      outs  [2*R, H] f32         : state_new slice.
    """
    if "indirect" in _nc_cache:
        return _nc_cache["indirect"]

    import concourse.bacc as bacc
    import concourse.bass as bass
    import concourse.mybir as mybir
    from concourse.tile import TileContext

    nc = bacc.Bacc("TRN2", target_bir_lowering=False)
    f32, i32 = mybir.dt.float32, mybir.dt.int32
    bslp = nc.dram_tensor("bslp", [R * GSPLIT, GLEN], f32, kind="ExternalInput")
    unaug = nc.dram_tensor("unaug", [R * USPLIT, ULEN], f32, kind="ExternalInput")
    state = nc.dram_tensor("state", [2 * R, H], f32, kind="ExternalInput")
    idxg = nc.dram_tensor("idxg", [R * GSPLIT, 1], i32, kind="ExternalInput")
    idxu = nc.dram_tensor("idxu", [UPASS_ROWS * USPLIT, NUPASS], i32, kind="ExternalInput")
    idxs = nc.dram_tensor("idxs", [2 * R, 1], i32, kind="ExternalInput")
    out = nc.dram_tensor("out", [R, (T + 1) * V], f32, kind="ExternalOutput")
    outs = nc.dram_tensor("outs", [2 * R, H], f32, kind="ExternalOutput")

    # out columns [0, T*V) viewed as [R, GSPLIT, GLEN] for chunked stores.
    outg3 = out[:, : T * V].rearrange("r (s c) -> r s c", c=GLEN)

    # One tile shape per pool: mixing tile shapes (differing partition
    # counts) in a single pool produced device-unrecoverable DMA crashes.
    with TileContext(nc) as tc:
        with (
            tc.tile_pool(name="gpool", bufs=KBUFS) as gpool,
            tc.tile_pool(name="upool", bufs=KUBUFS) as upool,
            tc.tile_pool(name="ig", bufs=1) as igpool,
            tc.tile_pool(name="iu", bufs=1) as iupool,
            tc.tile_pool(name="is", bufs=1) as ispool,
            tc.tile_pool(name="st", bufs=1) as stpool,
        ):
            idxg_sb = igpool.tile([R * GSPLIT, 1], i32)
            nc.sync.dma_start(out=idxg_sb[:], in_=idxg[:])
            idxu_sb = iupool.tile([UPASS_ROWS * USPLIT, NUPASS], i32)
            nc.sync.dma_start(out=idxu_sb[:], in_=idxu[:])
            idxs_sb = ispool.tile([2 * R, 1], i32)
            nc.sync.dma_start(out=idxs_sb[:], in_=idxs[:])

            # state gather: 40 rows of H floats.
            st = stpool.tile([2 * R, H], f32)
            nc.gpsimd.indirect_dma_start(
                out=st[:], out_offset=None, in_=state[:],
                in_offset=bass.IndirectOffsetOnAxis(ap=idxs_sb[:, :1], axis=0),
            )
            nc.sync.dma_start(out=outs[:], in_=st[:])

            # beam_seq_logprobs gather: 120 sub-rows x GLEN, chunked.
            for lo in range(0, GLEN, CHUNK):
                hi = min(GLEN, lo + CHUNK)
                w = hi - lo
                tile = gpool.tile([R * GSPLIT, CHUNK], f32, tag="g")
                nc.gpsimd.indirect_dma_start(
                    out=tile[:, :w], out_offset=None, in_=bslp[:],
                    in_offset=bass.IndirectOffsetOnAxis(ap=idxg_sb[:, :1], axis=0),
                    element_offset=lo,
                )
                nc.sync.dma_start(out=outg3[:, :, lo:hi], in_=tile[:, :w])

            # unaug gather -> out columns [T*V, (T+1)*V), 4 output rows per pass.
            for p in range(NUPASS):
                tu = upool.tile([UPASS_ROWS * USPLIT, ULEN], f32, tag="u")
                nc.gpsimd.indirect_dma_start(
                    out=tu[:], out_offset=None, in_=unaug[:],
                    in_offset=bass.IndirectOffsetOnAxis(ap=idxu_sb[:, p : p + 1], axis=0),
                )
                r0 = p * UPASS_ROWS
                nc.sync.dma_start(
                    out=out[r0 : r0 + UPASS_ROWS, T * V :], in_=tu[:]
                )
    nc.finalize()
    _nc_cache["indirect"] = nc
    return nc


def _build_device_program_hwdge():
    """v3: all gathers as HWDGE dma_start with register-based dynamic source
    offsets (value_load from a [1, R] int32 row-index tile on the SP engine).

    Rationale: SWDGE indirect-DMA descriptors cluster on SDMA engines 0-9 in
    this kernel (observed in traces), capping gather bandwidth at ~10/16
    engines. HWDGE descriptors split evenly by partition across all 16
    engines. Loads issue on nc.sync (SP HWDGE ring), stores on nc.scalar
    (ACT HWDGE ring) so load issue never blocks behind store waits.

    Per-core I/O:
      bslp  [12061680] f32 : core's beam_seq_logprobs slice, flat.
      unaug [20*V] f32     : core's unaug_logprobs slice, flat.
      state [2*R, H] f32   : core's state slice.
      idxr  [1, R] i32     : selected source row (local, in [0,R)) per output row.
      idxs  [2*R, 1] i32   : state gather rows (kept on indirect DMA: one op).
      out   [R, 13*V] f32, outs [2*R, H] f32.
    """
    if "hwdge" in _nc_cache:
        return _nc_cache["hwdge"]

    import concourse.bacc as bacc
    import concourse.bass as bass
    import concourse.mybir as mybir
    from concourse.tile import TileContext

    nc = bacc.Bacc("TRN2", target_bir_lowering=False)
    f32, i32 = mybir.dt.float32, mybir.dt.int32
    bslp = nc.dram_tensor("bslp", [BC * K * T * V], f32, kind="ExternalInput")
    unaug = nc.dram_tensor("unaug", [R * V], f32, kind="ExternalInput")
    state = nc.dram_tensor("state", [2 * R, H], f32, kind="ExternalInput")
    idxr = nc.dram_tensor("idxr", [1, R], i32, kind="ExternalInput")
    idxs = nc.dram_tensor("idxs", [2 * R, 1], i32, kind="ExternalInput")
    out = nc.dram_tensor("out", [R, (T + 1) * V], f32, kind="ExternalOutput")
    outs = nc.dram_tensor("outs", [2 * R, H], f32, kind="ExternalOutput")

    RL = T * V  # gathered row length (603084)

    with TileContext(nc) as tc:
        with (
            tc.tile_pool(name="gpool", bufs=KGBUFS) as gpool,
            tc.tile_pool(name="g2", bufs=3) as g2pool,
            tc.tile_pool(name="gt", bufs=3) as gtpool,
            tc.tile_pool(name="up", bufs=3) as upool,
            tc.tile_pool(name="ut", bufs=3) as utpool,
            tc.tile_pool(name="ix", bufs=1) as ixpool,
            tc.tile_pool(name="is2", bufs=1) as ispool,
            tc.tile_pool(name="st", bufs=1) as stpool,
        ):
            idx_sb = ixpool.tile([1, R], i32)
            nc.sync.dma_start(out=idx_sb[:], in_=idxr[:])
            idxs_sb = ispool.tile([2 * R, 1], i32)
            nc.sync.dma_start(out=idxs_sb[:], in_=idxs[:])

            # state gather: one indirect op, 40 rows of H floats.
            st = stpool.tile([2 * R, H], f32)
            nc.gpsimd.indirect_dma_start(
                out=st[:], out_offset=None, in_=state[:],
                in_offset=bass.IndirectOffsetOnAxis(ap=idxs_sb[:, :1], axis=0),
            )
            nc.scalar.dma_start(out=outs[:], in_=st[:])

            for r in range(R):
                row = nc.sync.value_load(idx_sb[0:1, r : r + 1])
                gbase = row * RL
                ubase = row * V
                # bslp history: 4x[128,1024] + [128,615] + [1,76]
                for c in range(NGC):
                    lo = c * 128 * GCP
                    tile = gpool.tile([128, GCP], f32, tag="g")
                    nc.sync.dma_start(
                        out=tile[:],
                        in_=bslp[bass.ds(gbase + lo, 128 * GCP)].rearrange(
                            "(p c) -> p c", c=GCP),
                    )
                    nc.scalar.dma_start(
                        out=out[r, lo : lo + 128 * GCP].rearrange(
                            "(p c) -> p c", c=GCP),
                        in_=tile[:],
                    )
                lo = NGC * 128 * GCP
                t2 = g2pool.tile([GR_P, GR_C], f32, tag="g2")
                nc.sync.dma_start(
                    out=t2[:],
                    in_=bslp[bass.ds(gbase + lo, GR_P * GR_C)].rearrange(
                        "(p c) -> p c", c=GR_C),
                )
                nc.scalar.dma_start(
                    out=out[r, lo : lo + GR_P * GR_C].rearrange(
                        "(p c) -> p c", c=GR_C),
                    in_=t2[:],
                )
                lo = RL - GR_TAIL
                tt = gtpool.tile([1, GR_TAIL], f32, tag="gt")
                nc.sync.dma_start(
                    out=tt[:],
                    in_=bslp[bass.ds(gbase + lo, GR_TAIL)].rearrange(
                        "(p c) -> p c", c=GR_TAIL),
                )
                nc.scalar.dma_start(out=out[r : r + 1, lo : lo + GR_TAIL], in_=tt[:])
                # unaug current step: [128,392] + [1,81] -> out cols [T*V, 13V)
                tu = upool.tile([128, UCP], f32, tag="u")
                nc.sync.dma_start(
                    out=tu[:],
                    in_=unaug[bass.ds(ubase, 128 * UCP)].rearrange(
                        "(p c) -> p c", c=UCP),
                )
                nc.scalar.dma_start(
                    out=out[r, RL : RL + 128 * UCP].rearrange("(p c) -> p c", c=UCP),
                    in_=tu[:],
                )
                tut = utpool.tile([1, U_TAIL], f32, tag="ut")
                nc.sync.dma_start(
                    out=tut[:],
                    in_=unaug[bass.ds(ubase + 128 * UCP, U_TAIL)].rearrange(
                        "(p c) -> p c", c=U_TAIL),
                )
                nc.scalar.dma_start(
                    out=out[r : r + 1, RL + 128 * UCP :], in_=tut[:])
    nc.finalize()
    _nc_cache["hwdge"] = nc
    return nc


def _reference_jax(logprobs, unaug_logprobs, beam_seq, beam_seq_logprobs,
                   beam_logprobs_sum, state, bdash):
    """Full eager-jax replication of the reference (fallback for unexpected
    problem instances)."""
    import jax
    import jax.numpy as jnp

    batch = beam_logprobs_sum.shape[0]
    vocab = logprobs.shape[-1]
    lp = jnp.asarray(logprobs).reshape(batch, -1, vocab)
    n_beams = lp.shape[1]
    cand = jnp.asarray(beam_logprobs_sum)[:, :, None] + lp
    flat = cand.reshape(batch, -1)
    ys, ix = jax.lax.top_k(flat, bdash)
    beam_ix = ix // vocab
    selected_ix = ix % vocab
    state_ix = (beam_ix + jnp.arange(batch)[:, None] * n_beams).reshape(-1)
    beam_seq_j = jnp.asarray(beam_seq)
    beam_seq_g = jnp.take_along_axis(beam_seq_j, beam_ix[:, :, None], axis=1)
    beam_seq_new = jnp.concatenate(
        [beam_seq_g, selected_ix[:, :, None].astype(beam_seq_j.dtype)], axis=-1)
    beam_logprobs_sum_new = (
        jnp.take_along_axis(jnp.asarray(beam_logprobs_sum), beam_ix, axis=1)
        + jnp.take_along_axis(lp.reshape(batch, -1), ix, axis=1))
    bslp_g = jnp.take_along_axis(
        jnp.asarray(beam_seq_logprobs), beam_ix[:, :, None, None], axis=1)
    beam_lp = jnp.take_along_axis(
        jnp.asarray(unaug_logprobs).reshape(batch, -1, vocab),
        beam_ix[:, :, None], axis=1)
    beam_seq_logprobs_new = jnp.concatenate(
        [bslp_g, beam_lp[:, :, None, :]], axis=2)
    state_new = jnp.asarray(state)[:, state_ix]
    return (np.asarray(beam_seq_new), np.asarray(beam_seq_logprobs_new),
            np.asarray(beam_logprobs_sum_new), np.asarray(state_new))


def kernel(logprobs, unaug_logprobs, beam_seq, beam_seq_logprobs,
           beam_logprobs_sum, state, bdash):
    logprobs = np.asarray(logprobs)
    unaug_logprobs = np.asarray(unaug_logprobs)
    beam_seq = np.asarray(beam_seq)
    beam_seq_logprobs = np.asarray(beam_seq_logprobs)
    beam_logprobs_sum = np.asarray(beam_logprobs_sum)
    state = np.asarray(state)
    bdash = int(bdash)

    expected_shape = (
        bdash == BDASH
        and logprobs.shape == (B * K, V)
        and unaug_logprobs.shape == (B * K, V)
        and beam_seq_logprobs.shape == (B, K, T, V)
        and beam_logprobs_sum.shape == (B, K)
        and state.shape[1:] == (B * K, H)
    )
    if not expected_shape:
        return _reference_jax(logprobs, unaug_logprobs, beam_seq,
                              beam_seq_logprobs, beam_logprobs_sum, state, bdash)

    # ---- selection + tiny outputs: the reference's own ops, verbatim -----
    # Using the identical eager jax.lax.top_k (on the process-default jax
    # device) guarantees the selected indices match the reference even where
    # the backend's top_k lowering has nonstandard index/tie semantics.
    import jax
    import jax.numpy as jnp

    lp = jnp.asarray(logprobs).reshape(B, K, V)
    sums_j = jnp.asarray(beam_logprobs_sum)
    cand = sums_j[:, :, None] + lp
    flat = cand.reshape(B, -1)
    _ys, ix = jax.lax.top_k(flat, BDASH)
    beam_ix = ix // V
    selected_ix = ix % V
    beam_seq_j = jnp.asarray(beam_seq)
    beam_seq_g = jnp.take_along_axis(beam_seq_j, beam_ix[:, :, None], axis=1)
    beam_seq_new = jnp.concatenate(
        [beam_seq_g, selected_ix[:, :, None].astype(beam_seq_j.dtype)], axis=-1)
    beam_logprobs_sum_new = (
        jnp.take_along_axis(sums_j, beam_ix, axis=1)
        + jnp.take_along_axis(lp.reshape(B, -1), ix, axis=1))

    # XLA gathers clamp out-of-range indices; replicate for the device gather.
    beam_ix_np = np.clip(np.asarray(beam_ix).astype(np.int64), 0, K - 1)

    # ---- device: all heavy gathers --------------------------------------
    from concourse.bass_utils import run_bass_kernel_spmd

    src_local = (np.arange(BC)[:, None] * K + beam_ix_np.reshape(M, BC, BDASH)
                 ).astype(np.int32)                    # [M, BC, 5] in [0, R)

    in_maps = []
    if KIMPL == "hwdge":
        nc = _build_device_program_hwdge()
        for m in range(M):
            sl = src_local[m].reshape(R)               # [20]
            idxs = np.concatenate([sl, R + sl]).reshape(-1, 1)
            in_maps.append({
                "bslp": beam_seq_logprobs[m * BC:(m + 1) * BC].reshape(-1),
                "unaug": unaug_logprobs[m * R:(m + 1) * R].reshape(-1),
                "state": state[:, m * R:(m + 1) * R, :].reshape(2 * R, H),
                "idxr": np.ascontiguousarray(sl.reshape(1, R)),
                "idxs": np.ascontiguousarray(idxs),
            })
    else:
        nc = _build_device_program()
        sub_g = np.arange(GSPLIT, dtype=np.int32)
        sub_u = np.arange(USPLIT, dtype=np.int32)
        for m in range(M):
            sl = src_local[m].reshape(R)               # [20]
            idxg = (sl[:, None] * GSPLIT + sub_g[None, :]).reshape(-1, 1)
            idxu = ((sl[:, None] * USPLIT + sub_u[None, :])
                    .reshape(NUPASS, UPASS_ROWS * USPLIT).T.copy())
            idxs = np.concatenate([sl, R + sl]).reshape(-1, 1)
            in_maps.append({
                "bslp": beam_seq_logprobs[m * BC:(m + 1) * BC].reshape(R * GSPLIT, GLEN),
                "unaug": unaug_logprobs[m * R:(m + 1) * R].reshape(R * USPLIT, ULEN),
                "state": state[:, m * R:(m + 1) * R, :].reshape(2 * R, H),
                "idxg": np.ascontiguousarray(idxg),
                "idxu": np.ascontiguousarray(idxu),
                "idxs": np.ascontiguousarray(idxs),
            })

    import os
    trace = bool(os.environ.get("KERNEL_TRACE"))
    res = run_bass_kernel_spmd(nc, in_maps, core_ids=list(range(M)), trace=trace)
    global last_results
    last_results = res

    beam_seq_logprobs_new = np.concatenate(
        [r["out"].reshape(BC, BDASH, T + 1, V) for r in res.results], axis=0)
    state_new = np.concatenate(
        [r["outs"].reshape(2, R, H) for r in res.results], axis=1)

    return (np.asarray(beam_seq_new), beam_seq_logprobs_new,
            np.asarray(beam_logprobs_sum_new), state_new)


# revision 11
# speedup vs baseline: 1.7282x; 1.1416x over previous
"""Trainium2 kernel for one beam_step (nn_CaptionModel): fused top-k + beam gather.

Problem (B=32, K=bdash=5, T=12, V=50257, H=512):
  - cand = beam_logprobs_sum[:,:,None] + logprobs.reshape(B,K,V); per batch row
    take the top-5 of the flattened (K*V) candidates.
  - Gather/concat the big per-beam history tensors along the beam dim with the
    selected beam indices, append the current step, and gather the state.

The memory-bound work (>99.9% of traffic: beam_seq_logprobs 386MB read +
beam_seq_logprobs_new 418MB write + unaug_logprobs + state) runs on 8
NeuronCores, data-parallel over the batch dim (4 batch rows per core), as a
single Bass/Tile SPMD program. Each core performs the data-dependent beam
gather with gpsimd indirect-DMA (DRAM->SBUF, offsets from a per-core int32
index tensor) followed by static HWDGE stores (SBUF->DRAM), streamed in ~4MB
chunks across 120 SBUF partitions.

The top-k / index-selection stage is O(B*K*V) compare work with a 5-element
result per row — negligible traffic next to the 870MB gather. It is computed
with the *same eager jax.lax.top_k op the reference uses*, so the selected
indices (including any backend-specific tie/padding semantics of top_k on the
current default jax device) match the reference bit-for-bit. The tiny outputs
(beam_seq_new ~13KB, beam_logprobs_sum_new 640B) are likewise produced by the
same jnp expressions as the reference; only the two large gathers
(beam_seq_logprobs_new, state_new) are offloaded to the Bass kernel.
"""

import sys

if "/opt/trn_rl_repo" not in sys.path:
    sys.path.insert(0, "/opt/trn_rl_repo")

import numpy as np

# Problem shape (hardcoded per spec nn_CaptionModel_47098611368401).
B, K, T, V, H = 32, 5, 12, 50257, 512
BDASH = 5
M = 8                     # cores
BC = B // M               # batch rows per core (4)
R = BC * K                # output rows per core (20)
GSPLIT = 6                # sub-rows per gathered beam row (12*V = 6 * GLEN)
GLEN = T * V // GSPLIT    # 100514
USPLIT = 29               # sub-rows per unaug row (V = 29 * 1733)
ULEN = V // USPLIT        # 1733
UPASS_ROWS = 4            # output rows per unaug pass (4*29=116 partitions)
NUPASS = R // UPASS_ROWS  # 5 passes
import os as _os
CHUNK = int(_os.environ.get("KCHUNK", "8192"))   # elements/partition per bslp chunk
KBUFS = int(_os.environ.get("KBUFS", "4"))       # gpool double-buffering depth
KUBUFS = int(_os.environ.get("KUBUFS", "2"))     # upool depth

KIMPL = _os.environ.get("KIMPL", "hwdge")         # "hwdge" | "indirect"
GCP = int(_os.environ.get("KGCP", "4711"))  # v3: elements/partition per bslp chunk
UCP = 392                 # v3: unaug chunk [128, 392] = 50176
U_TAIL = V - 128 * UCP            # 81
KGBUFS = int(_os.environ.get("KGBUFS", "6"))      # v3 gpool depth


def _row_plan(rl, gcp):
    """Cover a contiguous row of rl elements with (offset, parts, width) DMAs."""
    chunks = []
    off = 0
    while rl - off >= 128 * gcp:
        chunks.append((off, 128, gcp))
        off += 128 * gcp
    rem = rl - off
    if rem >= 128:
        w = rem // 128
        chunks.append((off, 128, w))
        off += 128 * w
        rem -= 128 * w
    if rem:
        chunks.append((off, 1, rem))
    return chunks

_nc_cache = {}
last_results = None


def _build_device_program():
    """Bass/Tile program: indirect-gather bslp/unaug/state rows, store to out.

    Per-core I/O (shapes chosen so every gathered unit is one DRAM row):
      bslp  [R*GSPLIT, GLEN] f32 : core's beam_seq_logprobs slice, rows are
                                   sixths of one (b,k) history [T*V].
      unaug [R*USPLIT, ULEN] f32 : core's unaug_logprobs slice, rows are
                                   29ths of one (b,k) vocab row [V].
      state [2*R, H] f32         : core's state slice, flattened (layer,row).
      idxg  [R*GSPLIT, 1] i32    : source row in bslp for each gathered sub-row.
      idxu  [UPASS_ROWS*USPLIT, NUPASS] i32 : per-pass source rows in unaug.
      idxs  [2*R, 1] i32         : source row in state for each output row.
      out   [R, 13*V] f32        : beam_seq_logprobs_new slice (row-major
                                   [bdash concat over b][T+1, V]).
      outs  [2*R, H] f32         : state_new slice.
    """
    if "indirect" in _nc_cache:
        return _nc_cache["indirect"]

    import concourse.bacc as bacc
    import concourse.bass as bass
    import concourse.mybir as mybir
    from concourse.tile import TileContext

    nc = bacc.Bacc("TRN2", target_bir_lowering=False)
    f32, i32 = mybir.dt.float32, mybir.dt.int32
    bslp = nc.dram_tensor("bslp", [R * GSPLIT, GLEN], f32, kind="ExternalInput")
    unaug = nc.dram_tensor("unaug", [R * USPLIT, ULEN], f32, kind="ExternalInput")
    state = nc.dram_tensor("state", [2 * R, H], f32, kind="ExternalInput")
    idxg = nc.dram_tensor("idxg", [R * GSPLIT, 1], i32, kind="ExternalInput")
    idxu = nc.dram_tensor("idxu", [UPASS_ROWS * USPLIT, NUPASS], i32, kind="ExternalInput")
    idxs = nc.dram_tensor("idxs", [2 * R, 1], i32, kind="ExternalInput")
    out = nc.dram_tensor("out", [R, (T + 1) * V], f32, kind="ExternalOutput")
    outs = nc.dram_tensor("outs", [2 * R, H], f32, kind="ExternalOutput")

    # out columns [0, T*V) viewed as [R, GSPLIT, GLEN] for chunked stores.
    outg3 = out[:, : T * V].rearrange("r (s c) -> r s c", c=GLEN)

    # One tile shape per pool: mixing tile shapes (differing partition
    # counts) in a single pool produced device-unrecoverable DMA crashes.
    with TileContext(nc) as tc:
        with (
            tc.tile_pool(name="gpool", bufs=KBUFS) as gpool,
            tc.tile_pool(name="upool", bufs=KUBUFS) as upool,
            tc.tile_pool(name="ig", bufs=1) as igpool,
            tc.tile_pool(name="iu", bufs=1) as iupool,
            tc.tile_pool(name="is", bufs=1) as ispool,
            tc.tile_pool(name="st", bufs=1) as stpool,
        ):
            idxg_sb = igpool.tile([R * GSPLIT, 1], i32)
            nc.sync.dma_start(out=idxg_sb[:], in_=idxg[:])
            idxu_sb = iupool.tile([UPASS_ROWS * USPLIT, NUPASS], i32)
            nc.sync.dma_start(out=idxu_sb[:], in_=idxu[:])
            idxs_sb = ispool.tile([2 * R, 1], i32)
            nc.sync.dma_start(out=idxs_sb[:], in_=idxs[:])

            # state gather: 40 rows of H floats.
            st = stpool.tile([2 * R, H], f32)
            nc.gpsimd.indirect_dma_start(
                out=st[:], out_offset=None, in_=state[:],
                in_offset=bass.IndirectOffsetOnAxis(ap=idxs_sb[:, :1], axis=0),
            )
            nc.sync.dma_start(out=outs[:], in_=st[:])

            # beam_seq_logprobs gather: 120 sub-rows x GLEN, chunked.
            for lo in range(0, GLEN, CHUNK):
                hi = min(GLEN, lo + CHUNK)
                w = hi - lo
                tile = gpool.tile([R * GSPLIT, CHUNK], f32, tag="g")
                nc.gpsimd.indirect_dma_start(
                    out=tile[:, :w], out_offset=None, in_=bslp[:],
                    in_offset=bass.IndirectOffsetOnAxis(ap=idxg_sb[:, :1], axis=0),
                    element_offset=lo,
                )
                nc.sync.dma_start(out=outg3[:, :, lo:hi], in_=tile[:, :w])

            # unaug gather -> out columns [T*V, (T+1)*V), 4 output rows per pass.
            for p in range(NUPASS):
                tu = upool.tile([UPASS_ROWS * USPLIT, ULEN], f32, tag="u")
                nc.gpsimd.indirect_dma_start(
                    out=tu[:], out_offset=None, in_=unaug[:],
                    in_offset=bass.IndirectOffsetOnAxis(ap=idxu_sb[:, p : p + 1], axis=0),
                )
                r0 = p * UPASS_ROWS
                nc.sync.dma_start(
                    out=out[r0 : r0 + UPASS_ROWS, T * V :], in_=tu[:]
                )
    nc.finalize()
    _nc_cache["indirect"] = nc
    return nc


def _build_device_program_hwdge():
    """v3: all gathers as HWDGE dma_start with register-based dynamic source
    offsets (value_load from a [1, R] int32 row-index tile on the SP engine).

    Rationale: SWDGE indirect-DMA descriptors cluster on SDMA engines 0-9 in
    this kernel (observed in traces), capping gather bandwidth at ~10/16
    engines. HWDGE descriptors split evenly by partition across all 16
    engines. Loads issue on nc.sync (SP HWDGE ring), stores on nc.scalar
    (ACT HWDGE ring) so load issue never blocks behind store waits.

    Per-core I/O:
      bslp  [12061680] f32 : core's beam_seq_logprobs slice, flat.
      unaug [20*V] f32     : core's unaug_logprobs slice, flat.
      state [2*R, H] f32   : core's state slice.
      idxr  [1, R] i32     : selected source row (local, in [0,R)) per output row.
      idxs  [2*R, 1] i32   : state gather rows (kept on indirect DMA: one op).
      out   [R, 13*V] f32, outs [2*R, H] f32.
    """
    if "hwdge" in _nc_cache:
        return _nc_cache["hwdge"]

    import concourse.bacc as bacc
    import concourse.bass as bass
    import concourse.mybir as mybir
    from concourse.tile import TileContext

    nc = bacc.Bacc("TRN2", target_bir_lowering=False)
    f32, i32 = mybir.dt.float32, mybir.dt.int32
    bslp = nc.dram_tensor("bslp", [BC * K * T * V], f32, kind="ExternalInput")
    unaug = nc.dram_tensor("unaug", [R * V], f32, kind="ExternalInput")
    state = nc.dram_tensor("state", [2 * R, H], f32, kind="ExternalInput")
    idxr = nc.dram_tensor("idxr", [1, R], i32, kind="ExternalInput")
    idxs = nc.dram_tensor("idxs", [2 * R, 1], i32, kind="ExternalInput")
    out = nc.dram_tensor("out", [R, (T + 1) * V], f32, kind="ExternalOutput")
    outs = nc.dram_tensor("outs", [2 * R, H], f32, kind="ExternalOutput")

    RL = T * V  # gathered row length (603084)

    with TileContext(nc) as tc:
        with (
            tc.tile_pool(name="gpool", bufs=KGBUFS) as gpool,
            tc.tile_pool(name="g2", bufs=3) as g2pool,
            tc.tile_pool(name="gt", bufs=3) as gtpool,
            tc.tile_pool(name="up", bufs=3) as upool,
            tc.tile_pool(name="ut", bufs=3) as utpool,
            tc.tile_pool(name="ix", bufs=1) as ixpool,
            tc.tile_pool(name="is2", bufs=1) as ispool,
            tc.tile_pool(name="st", bufs=1) as stpool,
        ):
            idx_sb = ixpool.tile([1, R], i32)
            nc.sync.dma_start(out=idx_sb[:], in_=idxr[:])
            idxs_sb = ispool.tile([2 * R, 1], i32)
            nc.sync.dma_start(out=idxs_sb[:], in_=idxs[:])

            # state gather: one indirect op, 40 rows of H floats.
            st = stpool.tile([2 * R, H], f32)
            nc.gpsimd.indirect_dma_start(
                out=st[:], out_offset=None, in_=state[:],
                in_offset=bass.IndirectOffsetOnAxis(ap=idxs_sb[:, :1], axis=0),
            )
            nc.scalar.dma_start(out=outs[:], in_=st[:])

            plan = _row_plan(RL, GCP)
            for r in range(R):
                row = nc.sync.value_load(idx_sb[0:1, r : r + 1])
                gbase = row * RL
                ubase = row * V
                for (lo, parts, width) in plan:
                    if parts == 128 and width == GCP:
                        tile = gpool.tile([128, GCP], f32, tag="g")
                    elif parts == 128:
                        tile = g2pool.tile([128, width], f32, tag="g2")
                    else:
                        tile = gtpool.tile([1, width], f32, tag="gt")
                    nc.sync.dma_start(
                        out=tile[:parts, :width],
                        in_=bslp[bass.ds(gbase + lo, parts * width)].rearrange(
                            "(p c) -> p c", c=width),
                    )
                    nc.scalar.dma_start(
                        out=out[r, lo : lo + parts * width].rearrange(
                            "(p c) -> p c", c=width),
                        in_=tile[:parts, :width],
                    )
                # unaug current step: [128,392] + [1,81] -> out cols [T*V, 13V)
                tu = upool.tile([128, UCP], f32, tag="u")
                nc.sync.dma_start(
                    out=tu[:],
                    in_=unaug[bass.ds(ubase, 128 * UCP)].rearrange(
                        "(p c) -> p c", c=UCP),
                )
                nc.scalar.dma_start(
                    out=out[r, RL : RL + 128 * UCP].rearrange("(p c) -> p c", c=UCP),
                    in_=tu[:],
                )
                tut = utpool.tile([1, U_TAIL], f32, tag="ut")
                nc.sync.dma_start(
                    out=tut[:],
                    in_=unaug[bass.ds(ubase + 128 * UCP, U_TAIL)].rearrange(
                        "(p c) -> p c", c=U_TAIL),
                )
                nc.scalar.dma_start(
                    out=out[r : r + 1, RL + 128 * UCP :], in_=tut[:])
    nc.finalize()
    _nc_cache["hwdge"] = nc
    return nc


def _reference_jax(logprobs, unaug_logprobs, beam_seq, beam_seq_logprobs,
                   beam_logprobs_sum, state, bdash):
    """Full eager-jax replication of the reference (fallback for unexpected
    problem instances)."""
    import jax
    import jax.numpy as jnp

    batch = beam_logprobs_sum.shape[0]
    vocab = logprobs.shape[-1]
    lp = jnp.asarray(logprobs).reshape(batch, -1, vocab)
    n_beams = lp.shape[1]
    cand = jnp.asarray(beam_logprobs_sum)[:, :, None] + lp
    flat = cand.reshape(batch, -1)
    ys, ix = jax.lax.top_k(flat, bdash)
    beam_ix = ix // vocab
    selected_ix = ix % vocab
    state_ix = (beam_ix + jnp.arange(batch)[:, None] * n_beams).reshape(-1)
    beam_seq_j = jnp.asarray(beam_seq)
    beam_seq_g = jnp.take_along_axis(beam_seq_j, beam_ix[:, :, None], axis=1)
    beam_seq_new = jnp.concatenate(
        [beam_seq_g, selected_ix[:, :, None].astype(beam_seq_j.dtype)], axis=-1)
    beam_logprobs_sum_new = (
        jnp.take_along_axis(jnp.asarray(beam_logprobs_sum), beam_ix, axis=1)
        + jnp.take_along_axis(lp.reshape(batch, -1), ix, axis=1))
    bslp_g = jnp.take_along_axis(
        jnp.asarray(beam_seq_logprobs), beam_ix[:, :, None, None], axis=1)
    beam_lp = jnp.take_along_axis(
        jnp.asarray(unaug_logprobs).reshape(batch, -1, vocab),
        beam_ix[:, :, None], axis=1)
    beam_seq_logprobs_new = jnp.concatenate(
        [bslp_g, beam_lp[:, :, None, :]], axis=2)
    state_new = jnp.asarray(state)[:, state_ix]
    return (np.asarray(beam_seq_new), np.asarray(beam_seq_logprobs_new),
            np.asarray(beam_logprobs_sum_new), np.asarray(state_new))


def kernel(logprobs, unaug_logprobs, beam_seq, beam_seq_logprobs,
           beam_logprobs_sum, state, bdash):
    logprobs = np.asarray(logprobs)
    unaug_logprobs = np.asarray(unaug_logprobs)
    beam_seq = np.asarray(beam_seq)
    beam_seq_logprobs = np.asarray(beam_seq_logprobs)
    beam_logprobs_sum = np.asarray(beam_logprobs_sum)
    state = np.asarray(state)
    bdash = int(bdash)

    expected_shape = (
        bdash == BDASH
        and logprobs.shape == (B * K, V)
        and unaug_logprobs.shape == (B * K, V)
        and beam_seq_logprobs.shape == (B, K, T, V)
        and beam_logprobs_sum.shape == (B, K)
        and state.shape[1:] == (B * K, H)
    )
    if not expected_shape:
        return _reference_jax(logprobs, unaug_logprobs, beam_seq,
                              beam_seq_logprobs, beam_logprobs_sum, state, bdash)

    # ---- selection + tiny outputs: the reference's own ops, verbatim -----
    # Using the identical eager jax.lax.top_k (on the process-default jax
    # device) guarantees the selected indices match the reference even where
    # the backend's top_k lowering has nonstandard index/tie semantics.
    import jax
    import jax.numpy as jnp

    lp = jnp.asarray(logprobs).reshape(B, K, V)
    sums_j = jnp.asarray(beam_logprobs_sum)
    cand = sums_j[:, :, None] + lp
    flat = cand.reshape(B, -1)
    _ys, ix = jax.lax.top_k(flat, BDASH)
    beam_ix = ix // V
    selected_ix = ix % V
    beam_seq_j = jnp.asarray(beam_seq)
    beam_seq_g = jnp.take_along_axis(beam_seq_j, beam_ix[:, :, None], axis=1)
    beam_seq_new = jnp.concatenate(
        [beam_seq_g, selected_ix[:, :, None].astype(beam_seq_j.dtype)], axis=-1)
    beam_logprobs_sum_new = (
        jnp.take_along_axis(sums_j, beam_ix, axis=1)
        + jnp.take_along_axis(lp.reshape(B, -1), ix, axis=1))

    # XLA gathers clamp out-of-range indices; replicate for the device gather.
    beam_ix_np = np.clip(np.asarray(beam_ix).astype(np.int64), 0, K - 1)

    # ---- device: all heavy gathers --------------------------------------
    from concourse.bass_utils import run_bass_kernel_spmd

    src_local = (np.arange(BC)[:, None] * K + beam_ix_np.reshape(M, BC, BDASH)
                 ).astype(np.int32)                    # [M, BC, 5] in [0, R)

    in_maps = []
    if KIMPL == "hwdge":
        nc = _build_device_program_hwdge()
        for m in range(M):
            sl = src_local[m].reshape(R)               # [20]
            idxs = np.concatenate([sl, R + sl]).reshape(-1, 1)
            in_maps.append({
                "bslp": beam_seq_logprobs[m * BC:(m + 1) * BC].reshape(-1),
                "unaug": unaug_logprobs[m * R:(m + 1) * R].reshape(-1),
                "state": state[:, m * R:(m + 1) * R, :].reshape(2 * R, H),
                "idxr": np.ascontiguousarray(sl.reshape(1, R)),
                "idxs": np.ascontiguousarray(idxs),
            })
    else:
        nc = _build_device_program()
        sub_g = np.arange(GSPLIT, dtype=np.int32)
        sub_u = np.arange(USPLIT, dtype=np.int32)
        for m in range(M):
            sl = src_local[m].reshape(R)               # [20]
            idxg = (sl[:, None] * GSPLIT + sub_g[None, :]).reshape(-1, 1)
            idxu = ((sl[:, None] * USPLIT + sub_u[None, :])
                    .reshape(NUPASS, UPASS_ROWS * USPLIT).T.copy())
            idxs = np.concatenate([sl, R + sl]).reshape(-1, 1)
            in_maps.append({
                "bslp": beam_seq_logprobs[m * BC:(m + 1) * BC].reshape(R * GSPLIT, GLEN),
                "unaug": unaug_logprobs[m * R:(m + 1) * R].reshape(R * USPLIT, ULEN),
                "state": state[:, m * R:(m + 1) * R, :].reshape(2 * R, H),
                "idxg": np.ascontiguousarray(idxg),
                "idxu": np.ascontiguousarray(idxu),
                "idxs": np.ascontiguousarray(idxs),
            })

    import os
    trace = bool(os.environ.get("KERNEL_TRACE"))
    res = run_bass_kernel_spmd(nc, in_maps, core_ids=list(range(M)), trace=trace)
    global last_results
    last_results = res

    beam_seq_logprobs_new = np.concatenate(
        [r["out"].reshape(BC, BDASH, T + 1, V) for r in res.results], axis=0)
    state_new = np.concatenate(
        [r["outs"].reshape(2, R, H) for r in res.results], axis=1)

    return (np.asarray(beam_seq_new), beam_seq_logprobs_new,
            np.asarray(beam_logprobs_sum_new), state_new)
